# revision 2
# baseline (speedup 1.0000x reference)
"""Distance-correlation (DisCo) loss kernel for Trainium2, sharded over 8 NeuronCores.

Math: reference computes NxN pairwise |vi-vj| matrices (a, b), weighted row
means, double centering, then scalar reductions.  Everything except the
genuinely 2-D term

    Q_ab[i] = sum_j w_j * |v1_i - v1_j| * |v2_i - v2_j|

has an exact O(N log N) closed form on the host (sorted prefix sums for
weighted |.| row sums, polynomial identities for squared terms).  The device
computes the 2-D part of Q_ab only (w == 1 fast path).

Device algorithm (sorted formulation): rows are sorted by v1 on the host, so
for any tile whose columns lie in 128-row blocks strictly below the tile's
row block, sign(v1_r - v1_c) is uniform and

    |v1_r - v1_c| * |v2_r - v2_c| = (v1_r - v1_c) * |v2_r - v2_c|.

Summing over rows r of a tile A[r, c] = |v2_r - v2_c| gives the per-column
masses  T1[c] = sum_r A[r, c]  and  Tv[c] = sum_r v1_r A[r, c], from which
the host reconstructs all one-sided sums via global closed forms.  So each
128x1024 tile costs ONE tensor_scalar prep (|v2_c - v2_r|, runs in DVE 4x
mode at fp16) plus ONE TensorE pass: the stationary operand is a multi-row
[ones | v1 | resid] matrix so plain and weighted column sums come out of the
same matmul, and fp8e4 tiles use DoubleRow perf mode (2 contraction-halves
per pass) at 0.5 cycles/row.  All matmuls write one shared PSUM bank through
sliding-window weight tensors (zero columns isolate bands), so the whole
output needs just two PSUM->SBUF copies and two DMAs.  Intra-128-block pairs
and all closed forms are computed on the host in float64.  Inputs are fp16-
quantized consistently (host and device); fp8 tile rounding adds unbiased
noise that averages out in the final scalar (measured rel err ~3e-4 vs the
2e-2 gate).
"""

import functools
import os

import numpy as np
import ml_dtypes

N = 8192
CORES = 8
B = 128            # row block
BD = 1024          # band width
ND = 8             # digits (row blocks per core)
NB = 8             # bands
VB = 7 * BD        # v2rep columns (bands 0..6; band 7 never a full-tile source)
POFF = [64 * m * (m - 1) for m in range(9)]  # vpart offsets, width m*128
PTOT = POFF[8]     # 3584

F8 = ml_dtypes.float8_e4m3

LAST_RESULT = None

# ---------------------------------------------------------------------------
# static plan: assignment of the 35 ops to (engine, dtype)
# engines: d16 = DVE fp16 prep (+fp16 PE), d8 = DVE fp8, p8 = Pool fp8,
# a8 = ACT fp8.  Greedy balance using the v1 CoreSim cost model.
# ---------------------------------------------------------------------------

_OPS = (
    # (kind, digit/slot, band, width); fulls: digit k covers bands 0..k-1
    [("full", k, j, 1024) for j in range(7) for k in range(j + 1, 8)]
    + [("part", m, None, m * 128) for m in range(1, 8)]
)

# modeled input-arrival times (ns) under the DMA ring plan:
#  SP: b0, b1, b2, vpartB(slots 1-4), b4 | ACT: scal, w16, w8, b6
#  Pool: b5, b3, vpartA(slots 5-7)
_ARR_BAND = {0: 2750, 1: 3540, 2: 4330, 3: 3700, 4: 5120, 5: 2860, 6: 5910}
_ARR_VA, _ARR_VB = 4460, 7890


def _op_ready(idx):
    kind, u, j, w = _OPS[idx]
    if kind == "full":
        return _ARR_BAND[j]
    return _ARR_VA if u >= 5 else _ARR_VB


# engine/dtype per op: legal abs producers only (ACT activation, DVE custom
# absdiff).  Alternate by modeled load: ACT 1038/KC vs DVE 1131/KC.
def _mk_plan():
    order = sorted(range(len(_OPS)), key=lambda i: (_op_ready(i), -_OPS[i][3]))
    la, ld = 0.0, 0.0
    plan = {}
    for i in order:
        w = _OPS[i][3]
        if la + 1.038 * w + 185 <= ld + 1.131 * w:
            plan[i] = ("act", 8)
            la += 1.038 * w + 185
        else:
            plan[i] = ("dve", 16)
            ld += 1.131 * w
    return [plan[i] for i in range(len(_OPS))]


_PLAN = _mk_plan()

_ORDER = sorted(range(len(_OPS)), key=lambda i: (_op_ready(i), -_OPS[i][3]))
_NWARM = 8


@functools.lru_cache(maxsize=1)
def _absdiff_op():
    """Custom fused DVE op: out = |in0 - s0| (walrus-legal via the runtime
    dve-op table, same registration pattern as the previous baseline)."""
    from operator import add

    import concourse.dve_ops as D
    from concourse.dve_spec import Spec, Src0, C0, Zero, maxx, lower
    from concourse.dve_uop import DveOpSpec

    d1 = Src0 - C0
    body = maxx(d1, Zero - d1)

    def ref(in0, in1, s0, s1, imm2):
        b = np.abs(in0.astype(np.float32) - s0).astype(np.float32)
        return b, b.reshape(b.shape[0], -1).sum(axis=-1, keepdims=True)

    spec = Spec(body=body, accum=add, accum_init=Zero, reference=ref)
    name = "DISCO_ABSDIFF"
    row = max(D._SUB_OPCODE_FOR_NAME.values()) + 1
    D._SUB_OPCODE_FOR_NAME[name] = row
    sha3 = DveOpSpec(
        name=name, opcode=row, uops=lower(spec, ver="v3"), rd1_en=True
    ).sha("v3")
    op = D.DveOp(name, spec, subdim=False, uops_sha={"v3": sha3})
    D.OPS.append(op)
    D.CUSTOM_DVE_SPECS[name] = spec
    return op


@functools.lru_cache(maxsize=1)
def _build_fast():
    import concourse.bacc as bacc
    import concourse.bass as bass
    import concourse.tile as tile
    from concourse import mybir

    f32 = mybir.dt.float32
    f16 = mybir.dt.float16
    f8 = mybir.dt.float8e4
    sub = mybir.AluOpType.subtract
    amax = mybir.AluOpType.abs_max
    mult = mybir.AluOpType.mult
    DR = mybir.MatmulPerfMode.DoubleRow

    nc = bacc.Bacc("TRN2", target_bir_lowering=False, debug=False)

    v2band_d = nc.dram_tensor("v2band", [VB], f16, kind="ExternalInput")
    vpart_d = nc.dram_tensor("vpart", [PTOT], f16, kind="ExternalInput")
    scal_d = nc.dram_tensor("scal", [128, 16], f32, kind="ExternalInput")
    w16_d = nc.dram_tensor("w16", [128, 16, 2, 48], f16, kind="ExternalInput")
    # fp8 arrays cannot cross the jax/axon boundary on TRN2; ship bytes as u8
    w8_d = nc.dram_tensor("w8", [128, 16, 2, 48], mybir.dt.uint8, kind="ExternalInput")
    obands_d = nc.dram_tensor("obands", [48, 512], f32, kind="ExternalOutput")
    opart_d = nc.dram_tensor("opart", [48, 448], f32, kind="ExternalOutput")

    def bcast(ap1d):
        return bass.AP(
            tensor=ap1d.tensor, offset=ap1d.offset, ap=[[0, 128]] + list(ap1d.ap)
        )

    # which op (in emission order) is the last matmul per bank
    last_full = max(i for i in _ORDER if _OPS[i][0] == "full")
    last_part = max(i for i in _ORDER if _OPS[i][0] == "part")
    # order by emission, find actual last emitted per kind
    for i in reversed(_ORDER):
        if _OPS[i][0] == "full":
            last_full = i
            break
    for i in reversed(_ORDER):
        if _OPS[i][0] == "part":
            last_part = i
            break

    with tile.TileContext(nc) as tc:
        with (
            tc.tile_pool(name="singles", bufs=1) as singles,
            tc.tile_pool(name="ab", bufs=36) as pab,
            tc.tile_pool(name="psum", bufs=1, space="PSUM") as ppsum,
        ):
            zerot = singles.tile([128, 512], f16)
            nc.vector.memset(zerot, 0.0)

            # --- input DMAs in the data-starved head window ---
            # SP ring: b0, b1, b2, b4, b6, vpartB
            sap = v2band_d.ap()
            vrep = {}
            for j in (0, 1, 2, 4, 6):
                t = singles.tile([128, BD], f16, tag=f"vb{j}")
                nc.sync.dma_start(out=t[:, :], in_=bcast(sap[j * BD:(j + 1) * BD]))
                vrep[j] = t
            vpB = singles.tile([128, POFF[5]], f16, tag="vpB")  # slots 1..4
            nc.sync.dma_start(out=vpB[:, :], in_=bcast(vpart_d.ap()[0:POFF[5]]))
            # ACT ring: scal, w16, w8 (all in the dead window)
            scal = singles.tile([128, 16], f32)
            nc.scalar.dma_start(out=scal[:, :], in_=scal_d.ap())
            w16 = singles.tile([128, 16, 2, 48], f16)
            nc.scalar.dma_start(out=w16[:, :, :, :], in_=w16_d.ap())
            w8 = singles.tile([128, 16, 2, 48], mybir.dt.uint8)
            nc.scalar.dma_start(out=w8[:, :, :, :], in_=w8_d.ap())
            # trigger the ACT table load now (off the critical path)
            atl = singles.tile([1, 1], f32, tag="atl")
            nc.scalar.activation(
                out=atl[:, :], in_=zerot[0:1, 0:1],
                func=mybir.ActivationFunctionType.Abs, bias=0.0, scale=1.0,
            )
            # Pool ring: b5, b3, vpartA
            for j in (5, 3):
                t = singles.tile([128, BD], f16, tag=f"vb{j}")
                nc.gpsimd.dma_start(out=t[:, :], in_=bcast(sap[j * BD:(j + 1) * BD]))
                vrep[j] = t
            vpA = singles.tile([128, PTOT - POFF[5]], f16, tag="vpA")  # slots 5..7
            nc.gpsimd.dma_start(out=vpA[:, :], in_=bcast(vpart_d.ap()[POFF[5]:PTOT]))

            # --- PE warmup (p-state ramp) + PSUM group starters ---
            scratch = ppsum.tile([1, 512], f32)
            for _ in range(_NWARM):
                nc.tensor.matmul(
                    scratch[:, :], zerot[:, 0:1], zerot[:, 0:512],
                    start=True, stop=True, skip_group_check=True,
                )
            bandsP = ppsum.tile([48, 512], f32, tag="bandsP")
            partP = ppsum.tile([48, 512], f32, tag="partP")
            nc.tensor.matmul(
                bandsP[:, :], zerot[:, 0:48], zerot[:, 0:512],
                start=True, stop=False, skip_group_check=True,
            )
            nc.tensor.matmul(
                partP[:, 0:448], zerot[:, 0:48], zerot[:, 0:448],
                start=True, stop=False, skip_group_check=True,
            )

            # --- main tile ops ---
            absdiff = _absdiff_op()

            def emit_prep(eng, dt, t, src, scol):
                s1 = scal[:, scol:scol + 1]
                if eng == "dve":
                    nc.vector._custom_dve(absdiff, out=t, in0=src, s0=s1)
                else:  # act
                    nc.scalar.activation(
                        out=t, in_=src,
                        func=mybir.ActivationFunctionType.Abs,
                        bias=s1, scale=-1.0,
                    )

            for idx in _ORDER:
                kind, u, j, w = _OPS[idx]
                eng, dt = _PLAN[idx]
                h = w // 2
                if kind == "full":
                    src = vrep[j][:, :]
                    scol, wu, pj = u, u, j
                    outP, ocols, stop = bandsP, 512, (idx == last_full)
                else:
                    m = u
                    if m >= 5:
                        src = vpA[:, POFF[m] - POFF[5]:POFF[m] - POFF[5] + w]
                    else:
                        src = vpB[:, POFF[m]:POFF[m] + w]
                    scol, wu, pj = 8 + m, 8 + m, m
                    outP, ocols, stop = partP, h, (idx == last_part)
                rows = 6 * pj + 6
                wlo = 42 - 6 * pj
                if dt == 16:
                    t = pab.tile([128, w], f16, tag="t16")
                    emit_prep(eng, 16, t[:, :], src, scol)
                    nc.tensor.matmul(
                        outP[0:rows, 0:ocols], w16[:, wu, 0, wlo:48], t[:, 0:h],
                        start=False, stop=False, skip_group_check=True,
                    )
                    nc.tensor.matmul(
                        outP[0:rows, 0:ocols], w16[:, wu, 1, wlo:48], t[:, h:w],
                        start=False, stop=stop, skip_group_check=True,
                    )
                else:
                    t = pab.tile([128, 2, h], f8, tag="t8")
                    emit_prep(eng, 8, t[:, :, :], src, scol)
                    nc.tensor.matmul(
                        outP[0:rows, 0:ocols], w8[:, wu, :, wlo:48].bitcast(f8), t[:, :, :],
                        start=False, stop=stop, perf_mode=DR,
                        skip_group_check=True,
                    )

            # --- drain PSUM and ship out ---
            sb_b = singles.tile([48, 512], f32, tag="sb_b")
            nc.scalar.copy(sb_b[:, :], bandsP[:, :])
            nc.sync.dma_start(out=obands_d.ap(), in_=sb_b[:, :])
            sb_p = singles.tile([48, 448], f32, tag="sb_p")
            nc.vector.tensor_scalar(sb_p[:, :], partP[:, 0:448], 1.0, None, mult)
            nc.gpsimd.dma_start(out=opart_d.ap(), in_=sb_p[:, :])

    nc.compile()
    return nc


# ---------------------------------------------------------------------------
# weighted fallback (general w) — identical to the previous baseline
# ---------------------------------------------------------------------------


@functools.lru_cache(maxsize=1)
def _build_weighted():
    import concourse.bacc as bacc
    import concourse.bass as bass
    import concourse.tile as tile
    from concourse import mybir

    f32 = mybir.dt.float32
    nc = bacc.Bacc("TRN2", target_bir_lowering=False, debug=False)

    JC = 2048
    NJC = N // JC
    BCH = 1024
    NIB = 8

    v1d = nc.dram_tensor("v1", [N], f32, kind="ExternalInput")
    v2d = nc.dram_tensor("v2", [N], f32, kind="ExternalInput")
    wd = nc.dram_tensor("w", [N], f32, kind="ExternalInput")
    vipackd = nc.dram_tensor("vipack", [128, 4 * NIB], f32, kind="ExternalInput")
    qabd = nc.dram_tensor("qab", [128, NIB], f32, kind="ExternalOutput")

    def bcast(ap1d):
        return bass.AP(
            tensor=ap1d.tensor, offset=ap1d.offset, ap=[[0, 128]] + list(ap1d.ap)
        )

    sub = mybir.AluOpType.subtract
    mult = mybir.AluOpType.mult
    add = mybir.AluOpType.add

    with tile.TileContext(nc) as tc:
        with (
            tc.tile_pool(name="singles", bufs=1) as singles,
            tc.tile_pool(name="ab", bufs=2) as pab,
            tc.tile_pool(name="scrap", bufs=1) as pscrap,
        ):
            v1rep = singles.tile([128, N], f32)
            v2rep = singles.tile([128, N], f32)
            wrep = singles.tile([128, N], f32)
            reps = [(v1rep, v1d), (v2rep, v2d), (wrep, wd)]
            for c in range(N // BCH):
                for rep, src in reps:
                    sap = src.ap()
                    nc.sync.dma_start(
                        out=rep[:, c * BCH:(c + 1) * BCH],
                        in_=bcast(sap[c * BCH:(c + 1) * BCH]),
                    )

            vipack = singles.tile([128, 4 * NIB], f32)
            nc.sync.dma_start(out=vipack[:, :], in_=vipackd.ap())
            vi1 = vipack[:, 0 * NIB:1 * NIB]
            nvi1 = vipack[:, 1 * NIB:2 * NIB]
            vi2 = vipack[:, 2 * NIB:3 * NIB]
            nvi2 = vipack[:, 3 * NIB:4 * NIB]

            qacc = singles.tile([128, NIB], f32)
            for ib in range(NIB):
                for jc in range(NJC):
                    j0 = jc * JC
                    ab = pab.tile([128, 2, JC], f32, tag="ab")
                    a = ab[:, 0, :]
                    b = ab[:, 1, :]
                    for t, (rep, vis, nvis) in enumerate(
                        ((v1rep, vi1, nvi1), (v2rep, vi2, nvi2))
                    ):
                        nc.scalar.activation(
                            out=ab[:, t, :],
                            in_=rep[:, j0:j0 + JC],
                            func=mybir.ActivationFunctionType.Abs,
                            bias=nvis[:, ib:ib + 1],
                            scale=1.0,
                        )
                    wb = pab.tile([128, JC], f32, tag="wb")
                    nc.vector.tensor_tensor(wb, b, wrep[:, j0:j0 + JC], mult)
                    scrap = pscrap.tile([128, JC], f32)
                    nc.vector.tensor_tensor(scrap, a, wb, mult)
                    nc.vector.tensor_scalar(
                        scrap,
                        scrap,
                        1.0,
                        (0.0 if jc == 0 else qacc[:, ib:ib + 1]),
                        mult,
                        add,
                        accum_out=qacc[:, ib:ib + 1],
                    )

            nc.sync.dma_start(out=qabd.ap(), in_=qacc[:, :])

    nc.compile()
    return nc


# ---------------------------------------------------------------------------
# SPMD runner (cached jit, identical approach to the previous baseline)
# ---------------------------------------------------------------------------


class _CachedRunner:
    def __init__(self, nc, n_cores=CORES):
        import jax
        from jax.experimental.shard_map import shard_map
        from jax.sharding import Mesh, PartitionSpec

        import concourse.mybir as mybir
        from concourse.bass2jax import (
            _bass_exec_p,
            install_neuronx_cc_hook,
            partition_id_tensor,
        )

        install_neuronx_cc_hook()
        self.n_cores = n_cores
        part_name = nc.partition_id_tensor.name if nc.partition_id_tensor else None
        in_names, out_names, out_avals, zero_outs = [], [], [], []
        for alloc in nc.m.functions[0].allocations:
            if not isinstance(alloc, mybir.MemoryLocationSet):
                continue
            name = alloc.memorylocations[0].name
            if alloc.kind == "ExternalInput":
                if name != part_name:
                    in_names.append(name)
            elif alloc.kind == "ExternalOutput":
                out_names.append(name)
                shape = tuple(alloc.tensor_shape)
                dtype = mybir.dt.np(alloc.dtype)
                out_avals.append(jax.core.ShapedArray(shape, dtype))
                zero_outs.append(np.zeros(shape, dtype))
        self.in_names, self.out_names = in_names, out_names
        self.zero_outs = zero_outs
        n_params = len(in_names)
        all_names = in_names + out_names
        if part_name is not None:
            all_names = all_names + [part_name]

        def _body(*args):
            operands = list(args)
            if part_name is not None:
                operands.append(partition_id_tensor())
            return tuple(
                _bass_exec_p.bind(
                    *operands,
                    out_avals=tuple(out_avals),
                    in_names=tuple(all_names),
                    out_names=tuple(out_names),
                    lowering_input_output_aliases=(),
                    sim_require_finite=True,
                    sim_require_nnan=True,
                    nc=nc,
                )
            )

        devices = jax.devices()[:n_cores]
        mesh = Mesh(np.asarray(devices), ("core",))
        nin = n_params + len(out_names)
        self.fn = jax.jit(
            shard_map(
                _body,
                mesh=mesh,
                in_specs=(PartitionSpec("core"),) * nin,
                out_specs=(PartitionSpec("core"),) * len(out_names),
                check_rep=False,
            ),
            donate_argnums=tuple(range(n_params, nin)),
            keep_unused=True,
        )

    def run(self, in_maps):
        n = self.n_cores
        concat_in = [
            np.concatenate([np.asarray(in_maps[c][k]) for c in range(n)], axis=0)
            for k in self.in_names
        ]
        concat_zero = [np.concatenate([z] * n, axis=0) for z in self.zero_outs]
        outs = [np.asarray(o) for o in self.fn(*concat_in, *concat_zero)]
        per_core = []
        for c in range(n):
            d = {}
            for k, o in zip(self.out_names, outs):
                m = o.shape[0] // n
                d[k] = o[c * m:(c + 1) * m]
            per_core.append(d)
        return per_core


_RUNNER_CACHE = {}


def _abs_weighted_sums(q, x):
    """out_i = sum_j q_j * |x_i - x_j|, exact via sorting (float64)."""
    o = np.argsort(x, kind="stable")
    xs, qs = x[o], x[o] * 0 + q[o]
    cq = np.cumsum(qs)
    cqx = np.cumsum(qs * xs)
    vals = xs * (2.0 * cq - cq[-1]) + cqx[-1] - 2.0 * cqx
    out = np.empty_like(vals)
    out[o] = vals
    return out


def _core_blocks(c):
    """Block index per digit k for core c, and partial-slot blocks."""
    digs = [8 * k + ((c + k) % 8) for k in range(8)]
    parts = [8 * ((m - c) % 8) + m for m in range(8)]  # slot m (m=0 unused)
    return digs, parts


def _make_in_map_fast(v1s32, v2s32, c):
    """v1s32/v2s32: SORTED fp16-quantized values as float32."""
    v2s16 = v2s32.astype(np.float16)
    digs, parts = _core_blocks(c)

    vpart = np.zeros(PTOT, np.float16)
    for m in range(1, 8):
        jstar = (m - c) % 8
        vpart[POFF[m]:POFF[m] + m * 128] = v2s16[jstar * BD: jstar * BD + m * 128]

    scal = np.zeros((128, 16), np.float32)
    w16 = np.zeros((128, 16, 2, 48), np.float16)
    w8 = np.zeros((128, 16, 2, 48), np.float32)
    for u in range(16):
        I = digs[u] if u < 8 else parts[u - 8]
        blk = slice(I * 128, (I + 1) * 128)
        scal[:, u] = v2s32[blk]
        v1b = v1s32[blk]
        v1q8 = v1b.astype(F8).astype(np.float32)
        res8 = ((v1b - v1q8) * 16.0).astype(F8).astype(np.float32)
        w16[:, u, 0, 42] = 1.0
        w16[:, u, 0, 44] = v1b
        w16[:, u, 1, 43] = 1.0
        w16[:, u, 1, 45] = v1b
        w8[:, u, 0, 42] = 1.0
        w8[:, u, 0, 44] = v1q8
        w8[:, u, 0, 46] = res8
        w8[:, u, 1, 43] = 1.0
        w8[:, u, 1, 45] = v1q8
        w8[:, u, 1, 47] = res8
    return {
        "v2band": np.ascontiguousarray(v2s16[:VB]),
        "vpart": vpart,
        "scal": scal,
        "w16": w16,
        "w8": w8.astype(F8).view(np.uint8),
    }


def _run_device_fast(v1s32, v2s32):
    """Returns (S1_above, Sv_above) float64 arrays over sorted rows."""
    global LAST_RESULT
    nc = _build_fast()
    in_maps = [_make_in_map_fast(v1s32, v2s32, c) for c in range(CORES)]
    trace = os.environ.get("DISCO_TRACE", "0") == "1"
    if trace or os.environ.get("DISCO_NO_RUNNER_CACHE", "0") == "1":
        from concourse.bass_utils import run_bass_kernel_spmd

        res = run_bass_kernel_spmd(
            nc, in_maps, core_ids=list(range(CORES)), trace=trace
        )
        LAST_RESULT = res
        results = res.results
    else:
        runner = _RUNNER_CACHE.get("fast")
        if runner is None:
            runner = _CachedRunner(nc)
            _RUNNER_CACHE["fast"] = runner
        results = runner.run(in_maps)

    S1 = np.zeros(N)
    Sv = np.zeros(N)
    for c, r in enumerate(results):
        ob = r["obands"].astype(np.float64)  # [48, 512]
        for j in range(7):
            base = j * BD
            S1[base:base + 512] += ob[6 * j + 0]
            S1[base + 512:base + 1024] += ob[6 * j + 1]
            Sv[base:base + 512] += ob[6 * j + 2] + ob[6 * j + 4] / 16.0
            Sv[base + 512:base + 1024] += ob[6 * j + 3] + ob[6 * j + 5] / 16.0
        op = r["opart"].astype(np.float64)  # [48, 448]
        for m in range(1, 8):
            jstar = (m - c) % 8
            base = jstar * BD
            h = m * 64
            S1[base:base + h] += op[6 * m + 0, :h]
            S1[base + h:base + 2 * h] += op[6 * m + 1, :h]
            Sv[base:base + h] += op[6 * m + 2, :h] + op[6 * m + 4, :h] / 16.0
            Sv[base + h:base + 2 * h] += op[6 * m + 3, :h] + op[6 * m + 5, :h] / 16.0
    return S1, Sv


def _qab_fast(v1q, v2q):
    """Q_ab for fp16-quantized inputs (float64 in/out), device-assisted."""
    order = np.argsort(v1q, kind="stable")
    v1s = v1q[order]
    v2s = v2q[order]

    S1_above, Sv_above = _run_device_fast(
        v1s.astype(np.float32), v2s.astype(np.float32)
    )

    # intra-block dense (f64)
    nb = N // B
    v1b = v1s.reshape(nb, B)
    v2b = v2s.reshape(nb, B)
    d1 = np.abs(v1b[:, :, None] - v1b[:, None, :])
    d2 = np.abs(v2b[:, :, None] - v2b[:, None, :])
    Qintra = (d1 * d2).sum(axis=2).reshape(N)
    S1_intra = d2.sum(axis=2).reshape(N)
    Sv_intra = (v1b[:, None, :] * d2).sum(axis=2).reshape(N)

    S1g = _abs_weighted_sums(np.ones(N), v2s)
    Svg = _abs_weighted_sums(v1s, v2s)

    E = v1s * (S1g - S1_intra - 2.0 * S1_above) + (
        2.0 * Sv_above - Svg + Sv_intra
    )
    Qs = Qintra + E
    out = np.empty(N)
    out[order] = Qs
    return out


def _make_in_map_weighted(v1, v2, w, c):
    ROWS = N // CORES
    NIB = ROWS // 128
    rows = v1[c * ROWS:(c + 1) * ROWS]
    rows2 = v2[c * ROWS:(c + 1) * ROWS]
    vr1 = np.ascontiguousarray(rows).reshape(NIB, 128).T
    vr2 = np.ascontiguousarray(rows2).reshape(NIB, 128).T
    return {
        "v1": v1,
        "v2": v2,
        "w": w,
        "vipack": np.ascontiguousarray(
            np.concatenate([vr1, -vr1, vr2, -vr2], axis=1)
        ),
    }


def _qab_weighted(v1, v2, w):
    global LAST_RESULT
    nc = _build_weighted()
    in_maps = [_make_in_map_weighted(v1, v2, w, c) for c in range(CORES)]
    trace = os.environ.get("DISCO_TRACE", "0") == "1"
    if trace or os.environ.get("DISCO_NO_RUNNER_CACHE", "0") == "1":
        from concourse.bass_utils import run_bass_kernel_spmd

        res = run_bass_kernel_spmd(
            nc, in_maps, core_ids=list(range(CORES)), trace=trace
        )
        LAST_RESULT = res
        results = res.results
    else:
        runner = _RUNNER_CACHE.get("weighted")
        if runner is None:
            runner = _CachedRunner(nc)
            _RUNNER_CACHE["weighted"] = runner
        results = runner.run(in_maps)
    ROWS = N // CORES
    parts = []
    for r in results:
        q = r["qab"].astype(np.float64)
        parts.append(q.T.reshape(ROWS))
    return np.concatenate(parts)


def kernel(var_1, var_2, normedweight, power):
    v1 = np.ascontiguousarray(np.asarray(var_1, dtype=np.float32))
    v2 = np.ascontiguousarray(np.asarray(var_2, dtype=np.float32))
    w = np.ascontiguousarray(np.asarray(normedweight, dtype=np.float32))
    p = int(np.asarray(power))
    ones = bool(np.all(w == np.float32(1.0)))

    if ones:
        # quantize consistently: device tiles are fp16-derived, so use the
        # same quantized values for all host-side closed forms
        v1 = v1.astype(np.float16).astype(np.float32)
        v2 = v2.astype(np.float16).astype(np.float32)
        qab = _qab_fast(v1.astype(np.float64), v2.astype(np.float64))
    else:
        qab = _qab_weighted(v1, v2, w)

    v1d, v2d, wd = v1.astype(np.float64), v2.astype(np.float64), w.astype(np.float64)
    u = _abs_weighted_sums(wd, v1d) / N
    v = _abs_weighted_sums(wd, v2d) / N
    W = wd.sum()
    ga = (wd * u).mean()
    gb = (wd * v).mean()
    al = u - ga
    be = v - gb
    Qaa = W * v1d**2 - 2.0 * v1d * (wd * v1d).sum() + (wd * v1d**2).sum()
    Qbb = W * v2d**2 - 2.0 * v2d * (wd * v2d).sum() + (wd * v2d**2).sum()
    Duu = (wd * u * u).sum()
    Duv = (wd * u * v).sum()
    Dvv = (wd * v * v).sum()
    Rawu = _abs_weighted_sums(wd * u, v1d)
    Rawv = _abs_weighted_sums(wd * v, v1d)
    Rbwu = _abs_weighted_sums(wd * u, v2d)
    Rbwv = _abs_weighted_sums(wd * v, v2d)

    k = 2.0 * N - W
    SAA = Qaa - 2.0 * Rawu + Duu - al**2 * k
    SBB = Qbb - 2.0 * Rbwv + Dvv - be**2 * k
    SAB = qab - Rawv - Rbwu + Duv - al * be * k

    num = (np.abs(SAB) / N * wd).mean()
    denA = (SAA / N * wd).mean()
    denB = (SBB / N * wd).mean()
    EPS = 1e-12
    with np.errstate(all="ignore"):
        if p == 1:
            d = np.abs(denA * denB)
            out = num / np.sqrt(d + EPS)
        elif p == 2:
            d = np.abs(denA * denB)
            out = num**2 / (d + EPS)
        else:
            out = (num / np.sqrt(denA * denB) + EPS) ** p
    if np.isnan(out):
        out = 0.0
    out = max(out, 0.0)
    return np.float32(out)


if __name__ == "__main__":
    print("plan clocks (modeled ns):", _PLAN_CLOCKS)
    from collections import Counter

    print(Counter(_PLAN))


# revision 4
# speedup vs baseline: 1.0438x; 1.0438x over previous
"""Distance-correlation (DisCo) loss kernel for Trainium2, sharded over 8 NeuronCores.

Math: reference computes NxN pairwise |vi-vj| matrices (a, b), weighted row
means, double centering, then scalar reductions.  Everything except the
genuinely 2-D term

    Q_ab[i] = sum_j w_j * |v1_i - v1_j| * |v2_i - v2_j|

has an exact O(N log N) closed form on the host (sorted prefix sums for
weighted |.| row sums, polynomial identities for squared terms).  The device
computes the 2-D part of Q_ab only (w == 1 fast path).

Device algorithm (sorted formulation): rows are sorted by v1 on the host, so
for any tile whose columns lie in 128-row blocks strictly below the tile's
row block, sign(v1_r - v1_c) is uniform and

    |v1_r - v1_c| * |v2_r - v2_c| = (v1_r - v1_c) * |v2_r - v2_c|.

Summing over rows r of a tile A[r, c] = |v2_r - v2_c| gives the per-column
masses  T1[c] = sum_r A[r, c]  and  Tv[c] = sum_r v1_r A[r, c], from which
the host reconstructs all one-sided sums via global closed forms.  So each
128x1024 tile costs ONE tensor_scalar prep (|v2_c - v2_r|, runs in DVE 4x
mode at fp16) plus ONE TensorE pass: the stationary operand is a multi-row
[ones | v1 | resid] matrix so plain and weighted column sums come out of the
same matmul, and fp8e4 tiles use DoubleRow perf mode (2 contraction-halves
per pass) at 0.5 cycles/row.  All matmuls write one shared PSUM bank through
sliding-window weight tensors (zero columns isolate bands), so the whole
output needs just two PSUM->SBUF copies and two DMAs.  Intra-128-block pairs
and all closed forms are computed on the host in float64.  Inputs are fp16-
quantized consistently (host and device); fp8 tile rounding adds unbiased
noise that averages out in the final scalar (measured rel err ~3e-4 vs the
2e-2 gate).
"""

import functools
import os

import numpy as np
import ml_dtypes

N = 8192
CORES = 8
B = 128            # row block
BD = 1024          # band width
ND = 8             # digits (row blocks per core)
NB = 8             # bands
VB = 7 * BD        # v2rep columns (bands 0..6; band 7 never a full-tile source)
POFF = [64 * m * (m - 1) for m in range(9)]  # vpart offsets, width m*128
PTOT = POFF[8]     # 3584

F8 = ml_dtypes.float8_e4m3

LAST_RESULT = None

# ---------------------------------------------------------------------------
# static plan: assignment of the 35 ops to (engine, dtype)
# engines: d16 = DVE fp16 prep (+fp16 PE), d8 = DVE fp8, p8 = Pool fp8,
# a8 = ACT fp8.  Greedy balance using the v1 CoreSim cost model.
# ---------------------------------------------------------------------------

_OPS = (
    # (kind, digit/slot, band, width); fulls: digit k covers bands 0..k-1
    [("full", k, j, 1024) for j in range(7) for k in range(j + 1, 8)]
    + [("part", m, None, m * 128) for m in range(1, 8)]
)

# modeled input-arrival times (ns) under the DMA ring plan:
#  SP: b0, b1, b2, vpartB(slots 1-4), b4 | ACT: scal, w16, w8, b6
#  Pool: b5, b3, vpartA(slots 5-7)
_ARR_BAND = {0: 2750, 1: 3540, 2: 4330, 3: 3700, 4: 5120, 5: 2860, 6: 5910}
_ARR_VA, _ARR_VB = 4460, 7890


def _op_ready(idx):
    kind, u, j, w = _OPS[idx]
    if kind == "full":
        return _ARR_BAND[j]
    return _ARR_VA if u >= 5 else _ARR_VB


# abs-channel per op (walrus-legal only): 'act' = ACT activation Abs (fp8
# out); 'custom' = fused DVE absdiff op (fp16 out); 'v1'/'v3' = relu
# decomposition |d| = max(d,0) - min(d,0) across DVE preps + Pool
# tensor_tensor, fp8 out.  From offline local-search on the cost model.
_PLAN = ['act', 'act', 'act', 'act', 'act', 'act', 'v1', 'v1', 'v1', 'act', 'v1', 'custom', 'v1', 'v3', 'act', 'v1', 'act', 'act', 'v3', 'custom', 'custom', 'act', 'act', 'v3', 'act', 'custom', 'act', 'v3', 'v3', 'act', 'v3', 'act', 'v1', 'custom', 'custom']

_ORDER = sorted(range(len(_OPS)), key=lambda i: (_op_ready(i), -_OPS[i][3]))
_NWARM = 8


@functools.lru_cache(maxsize=1)
def _absdiff_op():
    """Custom fused DVE op: out = |in0 - s0| (walrus-legal via the runtime
    dve-op table, same registration pattern as the previous baseline)."""
    from operator import add

    import concourse.dve_ops as D
    from concourse.dve_spec import Spec, Src0, C0, Zero, maxx, lower
    from concourse.dve_uop import DveOpSpec

    d1 = Src0 - C0
    body = maxx(d1, Zero - d1)

    def ref(in0, in1, s0, s1, imm2):
        b = np.abs(in0.astype(np.float32) - s0).astype(np.float32)
        return b, b.reshape(b.shape[0], -1).sum(axis=-1, keepdims=True)

    spec = Spec(body=body, accum=add, accum_init=Zero, reference=ref)
    name = "DISCO_ABSDIFF"
    row = max(D._SUB_OPCODE_FOR_NAME.values()) + 1
    D._SUB_OPCODE_FOR_NAME[name] = row
    sha3 = DveOpSpec(
        name=name, opcode=row, uops=lower(spec, ver="v3"), rd1_en=True
    ).sha("v3")
    op = D.DveOp(name, spec, subdim=False, uops_sha={"v3": sha3})
    D.OPS.append(op)
    D.CUSTOM_DVE_SPECS[name] = spec
    return op


@functools.lru_cache(maxsize=1)
def _build_fast():
    import concourse.bacc as bacc
    import concourse.bass as bass
    import concourse.tile as tile
    from concourse import mybir

    f32 = mybir.dt.float32
    f16 = mybir.dt.float16
    f8 = mybir.dt.float8e4
    sub = mybir.AluOpType.subtract
    amax = mybir.AluOpType.abs_max
    mult = mybir.AluOpType.mult
    add = mybir.AluOpType.add
    DR = mybir.MatmulPerfMode.DoubleRow

    nc = bacc.Bacc("TRN2", target_bir_lowering=False, debug=False)

    v2band_d = nc.dram_tensor("v2band", [VB], f16, kind="ExternalInput")
    vpart_d = nc.dram_tensor("vpart", [PTOT], f16, kind="ExternalInput")
    scal_d = nc.dram_tensor("scal", [128, 16], f32, kind="ExternalInput")
    w16_d = nc.dram_tensor("w16", [128, 16, 2, 48], f16, kind="ExternalInput")
    # fp8 arrays cannot cross the jax/axon boundary on TRN2; ship bytes as u8
    w8_d = nc.dram_tensor("w8", [128, 16, 2, 48], mybir.dt.uint8, kind="ExternalInput")
    obands_d = nc.dram_tensor("obands", [48, 512], f32, kind="ExternalOutput")
    opart_d = nc.dram_tensor("opart", [48, 448], f32, kind="ExternalOutput")

    def bcast(ap1d):
        return bass.AP(
            tensor=ap1d.tensor, offset=ap1d.offset, ap=[[0, 128]] + list(ap1d.ap)
        )

    # which op (in emission order) is the last matmul per bank
    last_full = max(i for i in _ORDER if _OPS[i][0] == "full")
    last_part = max(i for i in _ORDER if _OPS[i][0] == "part")
    # order by emission, find actual last emitted per kind
    for i in reversed(_ORDER):
        if _OPS[i][0] == "full":
            last_full = i
            break
    for i in reversed(_ORDER):
        if _OPS[i][0] == "part":
            last_part = i
            break

    with tile.TileContext(nc) as tc:
        with (
            tc.tile_pool(name="singles", bufs=1) as singles,
            tc.tile_pool(name="ab", bufs=8) as pab,
            tc.tile_pool(name="psum", bufs=1, space="PSUM") as ppsum,
        ):
            zerot = singles.tile([128, 512], f16)
            nc.vector.memset(zerot, 0.0)

            # --- input DMAs in the data-starved head window ---
            # SP ring: b0, b1, b2, b4, b6, vpartB
            sap = v2band_d.ap()
            vrep = {}
            for j in (0, 1, 2, 4, 6):
                t = singles.tile([128, BD], f16, tag=f"vb{j}")
                nc.sync.dma_start(out=t[:, :], in_=bcast(sap[j * BD:(j + 1) * BD]))
                vrep[j] = t
            vpB = singles.tile([128, POFF[5]], f16, tag="vpB")  # slots 1..4
            nc.sync.dma_start(out=vpB[:, :], in_=bcast(vpart_d.ap()[0:POFF[5]]))
            # ACT ring: scal, w16, w8 (all in the dead window)
            scal = singles.tile([128, 16], f32)
            nc.scalar.dma_start(out=scal[:, :], in_=scal_d.ap())
            w16 = singles.tile([128, 16, 2, 48], f16)
            nc.scalar.dma_start(out=w16[:, :, :, :], in_=w16_d.ap())
            w8 = singles.tile([128, 16, 2, 48], mybir.dt.uint8)
            nc.scalar.dma_start(out=w8[:, :, :, :], in_=w8_d.ap())
            # trigger the ACT table load now (off the critical path)
            atl = singles.tile([1, 1], f32, tag="atl")
            nc.scalar.activation(
                out=atl[:, :], in_=zerot[0:1, 0:1],
                func=mybir.ActivationFunctionType.Abs, bias=0.0, scale=1.0,
            )
            # Pool ring: b5, b3, vpartA
            for j in (5, 3):
                t = singles.tile([128, BD], f16, tag=f"vb{j}")
                nc.gpsimd.dma_start(out=t[:, :], in_=bcast(sap[j * BD:(j + 1) * BD]))
                vrep[j] = t
            vpA = singles.tile([128, PTOT - POFF[5]], f16, tag="vpA")  # slots 5..7
            nc.gpsimd.dma_start(out=vpA[:, :], in_=bcast(vpart_d.ap()[POFF[5]:PTOT]))

            # --- PE warmup (p-state ramp) + PSUM group starters ---
            scratch = ppsum.tile([1, 512], f32)
            for _ in range(_NWARM):
                nc.tensor.matmul(
                    scratch[:, :], zerot[:, 0:1], zerot[:, 0:512],
                    start=True, stop=True, skip_group_check=True,
                )
            bandsP = ppsum.tile([48, 512], f32, tag="bandsP")
            partP = ppsum.tile([48, 512], f32, tag="partP")
            nc.tensor.matmul(
                bandsP[:, :], zerot[:, 0:48], zerot[:, 0:512],
                start=True, stop=False, skip_group_check=True,
            )
            nc.tensor.matmul(
                partP[:, 0:448], zerot[:, 0:48], zerot[:, 0:448],
                start=True, stop=False, skip_group_check=True,
            )

            # --- main tile ops ---
            absdiff = _absdiff_op()
            mn = mybir.AluOpType.min
            mx = mybir.AluOpType.max

            def emit_prep(ch, t, src, scol, w):
                s1 = scal[:, scol:scol + 1]
                if ch == "custom":
                    nc.vector._custom_dve(absdiff, out=t, in0=src, s0=s1)
                elif ch == "act":
                    nc.scalar.activation(
                        out=t, in_=src,
                        func=mybir.ActivationFunctionType.Abs,
                        bias=s1, scale=-1.0,
                    )
                else:  # v1 / v3: |d| = max(d,0) - min(d,0)
                    d = pab.tile([128, w], f16, tag="d")
                    nc.vector.tensor_scalar(d[:, :], src, s1, None, sub)
                    r = pab.tile([128, w], f16, tag="r")
                    nc.vector.tensor_scalar(r[:, :], d[:, :], 0.0, None, mx)
                    if ch == "v3":
                        m = pab.tile([128, w], f16, tag="m")
                        nc.vector.tensor_scalar(m[:, :], d[:, :], 0.0, None, mn)
                        nc.gpsimd.tensor_tensor(t, r[:, :], m[:, :], sub)
                    else:  # v1: m = r - d on Pool, then t = r + m
                        m = pab.tile([128, w], f16, tag="m")
                        nc.gpsimd.tensor_tensor(m[:, :], r[:, :], d[:, :], sub)
                        nc.gpsimd.tensor_tensor(t, r[:, :], m[:, :], add)

            for idx in _ORDER:
                kind, u, j, w = _OPS[idx]
                ch = _PLAN[idx]
                dt = 16 if ch == "custom" else 8
                h = w // 2
                if kind == "full":
                    src = vrep[j][:, :]
                    scol, wu, pj = u, u, j
                    outP, ocols, stop = bandsP, 512, (idx == last_full)
                else:
                    m = u
                    if m >= 5:
                        src = vpA[:, POFF[m] - POFF[5]:POFF[m] - POFF[5] + w]
                    else:
                        src = vpB[:, POFF[m]:POFF[m] + w]
                    scol, wu, pj = 8 + m, 8 + m, m
                    outP, ocols, stop = partP, h, (idx == last_part)
                rows = 6 * pj + 6
                wlo = 42 - 6 * pj
                if dt == 16:
                    t = pab.tile([128, w], f16, tag="t16")
                    emit_prep(ch, t[:, :], src, scol, w)
                    nc.tensor.matmul(
                        outP[0:rows, 0:ocols], w16[:, wu, 0, wlo:48], t[:, 0:h],
                        start=False, stop=False, skip_group_check=True,
                    )
                    nc.tensor.matmul(
                        outP[0:rows, 0:ocols], w16[:, wu, 1, wlo:48], t[:, h:w],
                        start=False, stop=stop, skip_group_check=True,
                    )
                else:
                    t = pab.tile([128, 2, h], f8, tag="t8")
                    emit_prep(ch, t[:, :, :], src, scol, w)
                    nc.tensor.matmul(
                        outP[0:rows, 0:ocols], w8[:, wu, :, wlo:48].bitcast(f8), t[:, :, :],
                        start=False, stop=stop, perf_mode=DR,
                        skip_group_check=True,
                    )

            # --- drain PSUM and ship out ---
            sb_b = singles.tile([48, 512], f32, tag="sb_b")
            nc.scalar.copy(sb_b[:, :], bandsP[:, :])
            nc.sync.dma_start(out=obands_d.ap(), in_=sb_b[:, :])
            sb_p = singles.tile([48, 448], f32, tag="sb_p")
            nc.vector.tensor_scalar(sb_p[:, :], partP[:, 0:448], 1.0, None, mult)
            nc.gpsimd.dma_start(out=opart_d.ap(), in_=sb_p[:, :])

    nc.compile()
    return nc


# ---------------------------------------------------------------------------
# weighted fallback (general w) — identical to the previous baseline
# ---------------------------------------------------------------------------


@functools.lru_cache(maxsize=1)
def _build_weighted():
    import concourse.bacc as bacc
    import concourse.bass as bass
    import concourse.tile as tile
    from concourse import mybir

    f32 = mybir.dt.float32
    nc = bacc.Bacc("TRN2", target_bir_lowering=False, debug=False)

    JC = 2048
    NJC = N // JC
    BCH = 1024
    NIB = 8

    v1d = nc.dram_tensor("v1", [N], f32, kind="ExternalInput")
    v2d = nc.dram_tensor("v2", [N], f32, kind="ExternalInput")
    wd = nc.dram_tensor("w", [N], f32, kind="ExternalInput")
    vipackd = nc.dram_tensor("vipack", [128, 4 * NIB], f32, kind="ExternalInput")
    qabd = nc.dram_tensor("qab", [128, NIB], f32, kind="ExternalOutput")

    def bcast(ap1d):
        return bass.AP(
            tensor=ap1d.tensor, offset=ap1d.offset, ap=[[0, 128]] + list(ap1d.ap)
        )

    sub = mybir.AluOpType.subtract
    mult = mybir.AluOpType.mult
    add = mybir.AluOpType.add

    with tile.TileContext(nc) as tc:
        with (
            tc.tile_pool(name="singles", bufs=1) as singles,
            tc.tile_pool(name="ab", bufs=2) as pab,
            tc.tile_pool(name="scrap", bufs=1) as pscrap,
        ):
            v1rep = singles.tile([128, N], f32)
            v2rep = singles.tile([128, N], f32)
            wrep = singles.tile([128, N], f32)
            reps = [(v1rep, v1d), (v2rep, v2d), (wrep, wd)]
            for c in range(N // BCH):
                for rep, src in reps:
                    sap = src.ap()
                    nc.sync.dma_start(
                        out=rep[:, c * BCH:(c + 1) * BCH],
                        in_=bcast(sap[c * BCH:(c + 1) * BCH]),
                    )

            vipack = singles.tile([128, 4 * NIB], f32)
            nc.sync.dma_start(out=vipack[:, :], in_=vipackd.ap())
            vi1 = vipack[:, 0 * NIB:1 * NIB]
            nvi1 = vipack[:, 1 * NIB:2 * NIB]
            vi2 = vipack[:, 2 * NIB:3 * NIB]
            nvi2 = vipack[:, 3 * NIB:4 * NIB]

            qacc = singles.tile([128, NIB], f32)
            for ib in range(NIB):
                for jc in range(NJC):
                    j0 = jc * JC
                    ab = pab.tile([128, 2, JC], f32, tag="ab")
                    a = ab[:, 0, :]
                    b = ab[:, 1, :]
                    for t, (rep, vis, nvis) in enumerate(
                        ((v1rep, vi1, nvi1), (v2rep, vi2, nvi2))
                    ):
                        nc.scalar.activation(
                            out=ab[:, t, :],
                            in_=rep[:, j0:j0 + JC],
                            func=mybir.ActivationFunctionType.Abs,
                            bias=nvis[:, ib:ib + 1],
                            scale=1.0,
                        )
                    wb = pab.tile([128, JC], f32, tag="wb")
                    nc.vector.tensor_tensor(wb, b, wrep[:, j0:j0 + JC], mult)
                    scrap = pscrap.tile([128, JC], f32)
                    nc.vector.tensor_tensor(scrap, a, wb, mult)
                    nc.vector.tensor_scalar(
                        scrap,
                        scrap,
                        1.0,
                        (0.0 if jc == 0 else qacc[:, ib:ib + 1]),
                        mult,
                        add,
                        accum_out=qacc[:, ib:ib + 1],
                    )

            nc.sync.dma_start(out=qabd.ap(), in_=qacc[:, :])

    nc.compile()
    return nc


# ---------------------------------------------------------------------------
# SPMD runner (cached jit, identical approach to the previous baseline)
# ---------------------------------------------------------------------------


class _CachedRunner:
    def __init__(self, nc, n_cores=CORES):
        import jax
        from jax.experimental.shard_map import shard_map
        from jax.sharding import Mesh, PartitionSpec

        import concourse.mybir as mybir
        from concourse.bass2jax import (
            _bass_exec_p,
            install_neuronx_cc_hook,
            partition_id_tensor,
        )

        install_neuronx_cc_hook()
        self.n_cores = n_cores
        part_name = nc.partition_id_tensor.name if nc.partition_id_tensor else None
        in_names, out_names, out_avals, zero_outs = [], [], [], []
        for alloc in nc.m.functions[0].allocations:
            if not isinstance(alloc, mybir.MemoryLocationSet):
                continue
            name = alloc.memorylocations[0].name
            if alloc.kind == "ExternalInput":
                if name != part_name:
                    in_names.append(name)
            elif alloc.kind == "ExternalOutput":
                out_names.append(name)
                shape = tuple(alloc.tensor_shape)
                dtype = mybir.dt.np(alloc.dtype)
                out_avals.append(jax.core.ShapedArray(shape, dtype))
                zero_outs.append(np.zeros(shape, dtype))
        self.in_names, self.out_names = in_names, out_names
        self.zero_outs = zero_outs
        n_params = len(in_names)
        all_names = in_names + out_names
        if part_name is not None:
            all_names = all_names + [part_name]

        def _body(*args):
            operands = list(args)
            if part_name is not None:
                operands.append(partition_id_tensor())
            return tuple(
                _bass_exec_p.bind(
                    *operands,
                    out_avals=tuple(out_avals),
                    in_names=tuple(all_names),
                    out_names=tuple(out_names),
                    lowering_input_output_aliases=(),
                    sim_require_finite=True,
                    sim_require_nnan=True,
                    nc=nc,
                )
            )

        devices = jax.devices()[:n_cores]
        mesh = Mesh(np.asarray(devices), ("core",))
        nin = n_params + len(out_names)
        self.fn = jax.jit(
            shard_map(
                _body,
                mesh=mesh,
                in_specs=(PartitionSpec("core"),) * nin,
                out_specs=(PartitionSpec("core"),) * len(out_names),
                check_rep=False,
            ),
            donate_argnums=tuple(range(n_params, nin)),
            keep_unused=True,
        )

    def run(self, in_maps):
        n = self.n_cores
        concat_in = [
            np.concatenate([np.asarray(in_maps[c][k]) for c in range(n)], axis=0)
            for k in self.in_names
        ]
        concat_zero = [np.concatenate([z] * n, axis=0) for z in self.zero_outs]
        outs = [np.asarray(o) for o in self.fn(*concat_in, *concat_zero)]
        per_core = []
        for c in range(n):
            d = {}
            for k, o in zip(self.out_names, outs):
                m = o.shape[0] // n
                d[k] = o[c * m:(c + 1) * m]
            per_core.append(d)
        return per_core


_RUNNER_CACHE = {}


def _abs_weighted_sums(q, x):
    """out_i = sum_j q_j * |x_i - x_j|, exact via sorting (float64)."""
    o = np.argsort(x, kind="stable")
    xs, qs = x[o], x[o] * 0 + q[o]
    cq = np.cumsum(qs)
    cqx = np.cumsum(qs * xs)
    vals = xs * (2.0 * cq - cq[-1]) + cqx[-1] - 2.0 * cqx
    out = np.empty_like(vals)
    out[o] = vals
    return out


def _core_blocks(c):
    """Block index per digit k for core c, and partial-slot blocks."""
    digs = [8 * k + ((c + k) % 8) for k in range(8)]
    parts = [8 * ((m - c) % 8) + m for m in range(8)]  # slot m (m=0 unused)
    return digs, parts


def _make_in_map_fast(v1s32, v2s32, c):
    """v1s32/v2s32: SORTED fp16-quantized values as float32."""
    v2s16 = v2s32.astype(np.float16)
    digs, parts = _core_blocks(c)

    vpart = np.zeros(PTOT, np.float16)
    for m in range(1, 8):
        jstar = (m - c) % 8
        vpart[POFF[m]:POFF[m] + m * 128] = v2s16[jstar * BD: jstar * BD + m * 128]

    scal = np.zeros((128, 16), np.float32)
    w16 = np.zeros((128, 16, 2, 48), np.float16)
    w8 = np.zeros((128, 16, 2, 48), np.float32)
    for u in range(16):
        I = digs[u] if u < 8 else parts[u - 8]
        blk = slice(I * 128, (I + 1) * 128)
        scal[:, u] = v2s32[blk]
        v1b = v1s32[blk]
        v1q8 = v1b.astype(F8).astype(np.float32)
        res8 = ((v1b - v1q8) * 16.0).astype(F8).astype(np.float32)
        w16[:, u, 0, 42] = 1.0
        w16[:, u, 0, 44] = v1b
        w16[:, u, 1, 43] = 1.0
        w16[:, u, 1, 45] = v1b
        w8[:, u, 0, 42] = 1.0
        w8[:, u, 0, 44] = v1q8
        w8[:, u, 0, 46] = res8
        w8[:, u, 1, 43] = 1.0
        w8[:, u, 1, 45] = v1q8
        w8[:, u, 1, 47] = res8
    return {
        "v2band": np.ascontiguousarray(v2s16[:VB]),
        "vpart": vpart,
        "scal": scal,
        "w16": w16,
        "w8": w8.astype(F8).view(np.uint8),
    }


def _run_device_fast(v1s32, v2s32):
    """Returns (S1_above, Sv_above) float64 arrays over sorted rows."""
    global LAST_RESULT
    nc = _build_fast()
    in_maps = [_make_in_map_fast(v1s32, v2s32, c) for c in range(CORES)]
    trace = os.environ.get("DISCO_TRACE", "0") == "1"
    if trace or os.environ.get("DISCO_NO_RUNNER_CACHE", "0") == "1":
        from concourse.bass_utils import run_bass_kernel_spmd

        res = run_bass_kernel_spmd(
            nc, in_maps, core_ids=list(range(CORES)), trace=trace
        )
        LAST_RESULT = res
        results = res.results
    else:
        runner = _RUNNER_CACHE.get("fast")
        if runner is None:
            runner = _CachedRunner(nc)
            _RUNNER_CACHE["fast"] = runner
        results = runner.run(in_maps)

    S1 = np.zeros(N)
    Sv = np.zeros(N)
    for c, r in enumerate(results):
        ob = r["obands"].astype(np.float64)  # [48, 512]
        for j in range(7):
            base = j * BD
            S1[base:base + 512] += ob[6 * j + 0]
            S1[base + 512:base + 1024] += ob[6 * j + 1]
            Sv[base:base + 512] += ob[6 * j + 2] + ob[6 * j + 4] / 16.0
            Sv[base + 512:base + 1024] += ob[6 * j + 3] + ob[6 * j + 5] / 16.0
        op = r["opart"].astype(np.float64)  # [48, 448]
        for m in range(1, 8):
            jstar = (m - c) % 8
            base = jstar * BD
            h = m * 64
            S1[base:base + h] += op[6 * m + 0, :h]
            S1[base + h:base + 2 * h] += op[6 * m + 1, :h]
            Sv[base:base + h] += op[6 * m + 2, :h] + op[6 * m + 4, :h] / 16.0
            Sv[base + h:base + 2 * h] += op[6 * m + 3, :h] + op[6 * m + 5, :h] / 16.0
    return S1, Sv


def _qab_fast(v1q, v2q):
    """Q_ab for fp16-quantized inputs (float64 in/out), device-assisted."""
    order = np.argsort(v1q, kind="stable")
    v1s = v1q[order]
    v2s = v2q[order]

    S1_above, Sv_above = _run_device_fast(
        v1s.astype(np.float32), v2s.astype(np.float32)
    )

    # intra-block dense (f64)
    nb = N // B
    v1b = v1s.reshape(nb, B)
    v2b = v2s.reshape(nb, B)
    d1 = np.abs(v1b[:, :, None] - v1b[:, None, :])
    d2 = np.abs(v2b[:, :, None] - v2b[:, None, :])
    Qintra = (d1 * d2).sum(axis=2).reshape(N)
    S1_intra = d2.sum(axis=2).reshape(N)
    Sv_intra = (v1b[:, None, :] * d2).sum(axis=2).reshape(N)

    S1g = _abs_weighted_sums(np.ones(N), v2s)
    Svg = _abs_weighted_sums(v1s, v2s)

    E = v1s * (S1g - S1_intra - 2.0 * S1_above) + (
        2.0 * Sv_above - Svg + Sv_intra
    )
    Qs = Qintra + E
    out = np.empty(N)
    out[order] = Qs
    return out


def _make_in_map_weighted(v1, v2, w, c):
    ROWS = N // CORES
    NIB = ROWS // 128
    rows = v1[c * ROWS:(c + 1) * ROWS]
    rows2 = v2[c * ROWS:(c + 1) * ROWS]
    vr1 = np.ascontiguousarray(rows).reshape(NIB, 128).T
    vr2 = np.ascontiguousarray(rows2).reshape(NIB, 128).T
    return {
        "v1": v1,
        "v2": v2,
        "w": w,
        "vipack": np.ascontiguousarray(
            np.concatenate([vr1, -vr1, vr2, -vr2], axis=1)
        ),
    }


def _qab_weighted(v1, v2, w):
    global LAST_RESULT
    nc = _build_weighted()
    in_maps = [_make_in_map_weighted(v1, v2, w, c) for c in range(CORES)]
    trace = os.environ.get("DISCO_TRACE", "0") == "1"
    if trace or os.environ.get("DISCO_NO_RUNNER_CACHE", "0") == "1":
        from concourse.bass_utils import run_bass_kernel_spmd

        res = run_bass_kernel_spmd(
            nc, in_maps, core_ids=list(range(CORES)), trace=trace
        )
        LAST_RESULT = res
        results = res.results
    else:
        runner = _RUNNER_CACHE.get("weighted")
        if runner is None:
            runner = _CachedRunner(nc)
            _RUNNER_CACHE["weighted"] = runner
        results = runner.run(in_maps)
    ROWS = N // CORES
    parts = []
    for r in results:
        q = r["qab"].astype(np.float64)
        parts.append(q.T.reshape(ROWS))
    return np.concatenate(parts)


def kernel(var_1, var_2, normedweight, power):
    v1 = np.ascontiguousarray(np.asarray(var_1, dtype=np.float32))
    v2 = np.ascontiguousarray(np.asarray(var_2, dtype=np.float32))
    w = np.ascontiguousarray(np.asarray(normedweight, dtype=np.float32))
    p = int(np.asarray(power))
    ones = bool(np.all(w == np.float32(1.0)))

    if ones:
        # quantize consistently: device tiles are fp16-derived, so use the
        # same quantized values for all host-side closed forms
        v1 = v1.astype(np.float16).astype(np.float32)
        v2 = v2.astype(np.float16).astype(np.float32)
        qab = _qab_fast(v1.astype(np.float64), v2.astype(np.float64))
    else:
        qab = _qab_weighted(v1, v2, w)

    v1d, v2d, wd = v1.astype(np.float64), v2.astype(np.float64), w.astype(np.float64)
    u = _abs_weighted_sums(wd, v1d) / N
    v = _abs_weighted_sums(wd, v2d) / N
    W = wd.sum()
    ga = (wd * u).mean()
    gb = (wd * v).mean()
    al = u - ga
    be = v - gb
    Qaa = W * v1d**2 - 2.0 * v1d * (wd * v1d).sum() + (wd * v1d**2).sum()
    Qbb = W * v2d**2 - 2.0 * v2d * (wd * v2d).sum() + (wd * v2d**2).sum()
    Duu = (wd * u * u).sum()
    Duv = (wd * u * v).sum()
    Dvv = (wd * v * v).sum()
    Rawu = _abs_weighted_sums(wd * u, v1d)
    Rawv = _abs_weighted_sums(wd * v, v1d)
    Rbwu = _abs_weighted_sums(wd * u, v2d)
    Rbwv = _abs_weighted_sums(wd * v, v2d)

    k = 2.0 * N - W
    SAA = Qaa - 2.0 * Rawu + Duu - al**2 * k
    SBB = Qbb - 2.0 * Rbwv + Dvv - be**2 * k
    SAB = qab - Rawv - Rbwu + Duv - al * be * k

    num = (np.abs(SAB) / N * wd).mean()
    denA = (SAA / N * wd).mean()
    denB = (SBB / N * wd).mean()
    EPS = 1e-12
    with np.errstate(all="ignore"):
        if p == 1:
            d = np.abs(denA * denB)
            out = num / np.sqrt(d + EPS)
        elif p == 2:
            d = np.abs(denA * denB)
            out = num**2 / (d + EPS)
        else:
            out = (num / np.sqrt(denA * denB) + EPS) ** p
    if np.isnan(out):
        out = 0.0
    out = max(out, 0.0)
    return np.float32(out)


if __name__ == "__main__":
    print("plan clocks (modeled ns):", _PLAN_CLOCKS)
    from collections import Counter

    print(Counter(_PLAN))
